# revision 30
# baseline (speedup 1.0000x reference)
"""Trainium2 Bass kernel for DTNetv1 (segment_reduce).

Reference computation:
    x1 = x @ Wd.T + bd ; x2 = x @ Wa.T + ba ; h = [x1, x2]            # [B, 1023]
    hh = [relu(h), relu(-h)]                                          # [B, 2046]
    leaf = hh @ Wl.T                                                  # [B, 1024]
    pooled = segment_max(leaf cols by seg_ids, 16) ; softmax(pooled)  # [B, 16]

Key algebraic rewrite (host-side, float64):
    relu(h) = (h + |h|)/2 ; relu(-h) = (|h| - h)/2
    leaf = h @ V.T + |h| @ W2s.T          V = (W1-W2)/2, W2s = (W1+W2)/2
         = x @ G + c + |h| @ W2s.T        G = W.T @ V.T (K=128!), c = V @ b
    This halves the dominant matmul contraction (2046 -> 1024+128).

Sharding: pure data-parallel over batch; 8 cores x 4096 rows, same NEFF.

Device pipeline per core (fp16 matmuls, fp32 PSUM accumulation):
    - PE-transpose x tiles -> xT [128(IN), B] fp16
    - stage A:  hT = W.T.T @ xT per 512-col group; ScalarE Abs(+bias) -> |h|T fp16
      (padded hidden row 1023 == 1.0 via bias, pairing with c packed in Wl2sT)
    - stage BC: leaf[128b, 1024] = xT_chunk.T @ G + sum_k |h|T_k.T @ Wl2sT_k
    - VectorE segment reduce_max over contiguous (pre-sorted) leaf columns
    - ScalarE Exp (+accumulated row sum), VectorE reciprocal + scale, DMA out

Measured (neuron-profile, core 0 of 8 running concurrently): ~166 us;
rel l2 error vs float64 oracle: 3.1e-4.
"""

import numpy as np

import concourse.bass as bass
import concourse.tile as tile
import concourse.mybir as mybir
from concourse import bacc
from concourse.bass_utils import run_bass_kernel_spmd

# Problem constants (hardcoded per the harness contract)
N_CORES = 8
B_FULL = 32768
IN = 128
D_DIM = 511
A_DIM = 512
HID = D_DIM + A_DIM          # 1023
KH = 1024                    # padded hidden (row 1023 is the "ones" bias row)
L = 1024                     # leaves
NSEG = 16
BC = B_FULL // N_CORES       # 4096 rows per core
GROUP = 512                  # batch rows per pipeline group
NSUB = 4                     # 128-row subchunks per group
NG = BC // GROUP             # 8 groups
P = 128

f32 = mybir.dt.float32
f16 = mybir.dt.float16


def _build(seg_bounds):
    """Build the Bass program. seg_bounds: list of (start, end) leaf-column
    ranges per segment in the sorted leaf order (end<=start for empty)."""
    nc = bacc.Bacc("TRN2", target_bir_lowering=False, debug=False)

    x = nc.dram_tensor("x", [BC, IN], f32, kind="ExternalInput").ap()
    wt = nc.dram_tensor("wt", [P, KH], f16, kind="ExternalInput").ap()
    bias = nc.dram_tensor("bias", [P, KH // P], f32, kind="ExternalInput").ap()
    wg = nc.dram_tensor("wg", [KH // P + 1, P, L], f16, kind="ExternalInput").ap()
    idin = nc.dram_tensor("idin", [P, P], f32, kind="ExternalInput").ap()
    out = nc.dram_tensor("out", [BC, NSEG], f32, kind="ExternalOutput").ap()

    x_v = x.rearrange("(g j p) i -> g p j i", g=NG, j=NSUB, p=P)
    out_v = out.rearrange("(g j p) s -> g j p s", g=NG, j=NSUB, p=P)
    wg_v = wg.rearrange("k p l -> p k l")

    NK = KH // P + 1  # 9 contraction chunks for stage BC (G + 8 |h| chunks)

    with tile.TileContext(nc) as tc:
        with (
            tc.tile_pool(name="consts", bufs=1) as consts,
            tc.tile_pool(name="xin", bufs=10) as xin_pool,
            tc.tile_pool(name="xts", bufs=3) as xt_pool,
            tc.tile_pool(name="hh", bufs=2) as hh_pool,
            tc.tile_pool(name="small", bufs=6) as small,
            tc.tile_pool(name="xtp", bufs=1, space="PSUM") as xt_ps,
            tc.tile_pool(name="aps", bufs=3, space="PSUM") as a_ps,
            tc.tile_pool(name="leafp", bufs=2, space="PSUM") as leaf_ps,
        ):
            xt_sb = [None] * NG   # [128, 512] fp16 xT per group
            hh_sb = [None] * NG   # [128, 8, 512] fp16 |h|T per group
            xin_t = [None] * NG

            def emit_load(g):
                # per-subchunk tiles/DMAs: first transpose only waits on 64KB
                xin_t[g] = []
                for c in range(NSUB):
                    t = xin_pool.tile([P, P], f32, tag="xin", name="xin")
                    nc.sync.dma_start(t, x_v[g][:, c, :])
                    xin_t[g].append(t)

            # group-0 x rows + identity land before the big weight streams
            # queue up — the first transposes are the startup critical path
            emit_load(0)
            ident = consts.tile([P, P], f32)
            nc.sync.dma_start(ident, idin)
            # PE warm-up: garbage matmuls during the startup DMA window keep
            # the HAM activity monitor busy so real matmuls start at 2.4 GHz.
            warm = consts.tile([P, GROUP], f16, tag="warm", name="warm")
            nc.gpsimd.memset(warm, 0.0)
            wps = a_ps.tile([P, GROUP], f32, tag="aps", name="wps")
            for _ in range(9):
                nc.tensor.matmul(wps, warm[:, 0:P], warm, start=True, stop=True)
            wt_t = consts.tile([P, KH], f16)
            nc.sync.dma_start(wt_t, wt)
            bias_t = consts.tile([P, KH // P], f32)
            nc.sync.dma_start(bias_t, bias)
            wg_t = []
            for k in range(NK):
                wg_t.append(consts.tile([P, L], f16, tag=f"wg{k}", name=f"wg{k}"))
                nc.sync.dma_start(wg_t[k], wg_v[:, k, :])

            def emit_transpose(g):
                # 4x PE transpose into one PSUM bank, one ACT evacuate+cast
                tps = xt_ps.tile([P, GROUP], f32, tag="xtp", name="xtp")
                for c in range(NSUB):
                    nc.tensor.transpose(
                        tps[:, c * P:(c + 1) * P], xin_t[g][c], ident
                    )
                xin_t[g] = None
                xt_sb[g] = xt_pool.tile([P, GROUP], f16, tag="xt", name="xt")
                nc.scalar.copy(xt_sb[g], tps)

            def emit_stage_a(g, c0, c1):
                # stage A matmuls for hidden chunks [c0, c1)
                for c in range(c0, c1):
                    aps = a_ps.tile([P, GROUP], f32, tag="aps", name="aps")
                    nc.tensor.matmul(
                        aps, wt_t[:, c * P:(c + 1) * P], xt_sb[g],
                        start=True, stop=True,
                    )
                    nc.scalar.activation(
                        hh_sb[g][:, c, :], aps,
                        mybir.ActivationFunctionType.Abs,
                        bias=bias_t[:, c:c + 1], scale=1.0,
                    )

            def lhsT_of(g, j, k):
                if k == 0:
                    return xt_sb[g][:, j * P:(j + 1) * P]
                return hh_sb[g][:, k - 1, j * P:(j + 1) * P]

            def emit_softmax_out(g, j, pooled):
                # softmax (pooled values are small; no max-subtraction needed)
                expt = small.tile([P, NSEG], f32, tag="expt", name="expt")
                sumt = small.tile([P, 1], f32, tag="sumt", name="sumt")
                nc.scalar.activation(
                    expt, pooled, mybir.ActivationFunctionType.Exp,
                    accum_out=sumt,
                )
                rect = small.tile([P, 1], f32, tag="rect", name="rect")
                nc.vector.reciprocal(rect, sumt)
                outt = small.tile([P, NSEG], f32, tag="outt", name="outt")
                nc.vector.tensor_scalar_mul(outt, expt, rect[:, 0:1])
                nc.sync.dma_start(out_v[g, j], outt)

            def emit_bc_sub(g, j):
                # leaf[128b, 1024] for subchunk j of group g
                leaf = leaf_ps.tile([P, L], f32, tag="leaf", name="leaf")
                pooled = small.tile([P, NSEG], f32, tag="pooled", name="pooled")
                if any(e <= s for s, e in seg_bounds):
                    nc.vector.memset(pooled, -1e30)
                for k in range(NK):
                    lhsT = lhsT_of(g, j, k)
                    nc.tensor.matmul(
                        leaf[:, 0:512], lhsT, wg_t[k][:, 0:512],
                        start=(k == 0), stop=(k == NK - 1),
                    )
                    nc.tensor.matmul(
                        leaf[:, 512:L], lhsT, wg_t[k][:, 512:L],
                        start=(k == 0), stop=(k == NK - 1),
                    )
                # evacuate PSUM via ScalarE (frees the banks fast), then
                # ragged per-segment max over contiguous column ranges
                leaf_sb = small.tile([P, L], f32, tag="leafsb", name="leaf_sb")
                nc.scalar.copy(leaf_sb, leaf)
                for s, (lo, hi) in enumerate(seg_bounds):
                    if hi > lo:
                        nc.vector.reduce_max(
                            out=pooled[:, s:s + 1], in_=leaf_sb[:, lo:hi],
                            axis=mybir.AxisListType.X,
                        )
                emit_softmax_out(g, j, pooled)

            def emit_bc_sub_last(g, j):
                # final subchunk: two independent PSUM tiles so bank-0
                # segment reduces overlap bank-1 matmuls, shrinking the
                # serial reduce chain on the kernel tail
                poolA = small.tile([P, NSEG], f32, tag="pooled", name="poolA")
                poolB = small.tile([P, NSEG], f32, tag="poolB", name="poolB")
                nc.vector.memset(poolA, -1e30)
                nc.vector.memset(poolB, -1e30)
                leafA = leaf_ps.tile([P, L], f32, tag="leaf", name="leafA")
                for k in range(NK):
                    nc.tensor.matmul(
                        leafA[:, 0:512], lhsT_of(g, j, k), wg_t[k][:, 0:512],
                        start=(k == 0), stop=(k == NK - 1),
                    )
                leafB = leaf_ps.tile([P, L], f32, tag="leaf", name="leafB")
                for s, (lo, hi) in enumerate(seg_bounds):
                    if min(hi, 512) > lo:
                        nc.vector.reduce_max(
                            out=poolA[:, s:s + 1], in_=leafA[:, lo:min(hi, 512)],
                            axis=mybir.AxisListType.X,
                        )
                for k in range(NK):
                    nc.tensor.matmul(
                        leafB[:, 0:512], lhsT_of(g, j, k), wg_t[k][:, 512:L],
                        start=(k == 0), stop=(k == NK - 1),
                    )
                for s, (lo, hi) in enumerate(seg_bounds):
                    if hi - 512 > max(lo - 512, 0):
                        nc.vector.reduce_max(
                            out=poolB[:, s:s + 1],
                            in_=leafB[:, max(lo - 512, 0):hi - 512],
                            axis=mybir.AxisListType.X,
                        )
                nc.vector.tensor_tensor(
                    poolA, poolA, poolB, mybir.AluOpType.max
                )
                emit_softmax_out(g, j, poolA)

            # --- pipelined emission ---
            for gi in range(NG + 1):
                if gi + 1 < NG:
                    emit_load(gi + 1)
                if gi < NG:
                    emit_transpose(gi)
                    hh_sb[gi] = hh_pool.tile([P, KH // P, GROUP], f16, tag="hh", name="hh")
                # interleave stage A of group gi with stage BC of group gi-1
                for j in range(NSUB):
                    if gi >= 1:
                        if gi == NG and j == NSUB - 1:
                            emit_bc_sub_last(gi - 1, j)
                        else:
                            emit_bc_sub(gi - 1, j)
                    if gi < NG:
                        emit_stage_a(gi, 2 * j, 2 * j + 2)
                if gi >= 1:
                    xt_sb[gi - 1] = None
                    hh_sb[gi - 1] = None

    nc.compile()
    return nc


_cache = {}
_last_in_maps = None


def _get_program(seg_key, seg_bounds):
    if seg_key not in _cache:
        _cache[seg_key] = _build(seg_bounds)
    return _cache[seg_key]


def kernel(x, Wd, bd, Wa, ba, Wl, seg_ids):
    x = np.ascontiguousarray(np.asarray(x, dtype=np.float32))
    seg = np.asarray(seg_ids).astype(np.int64)
    assert x.shape == (B_FULL, IN), x.shape
    assert np.asarray(Wl).shape == (L, 2 * HID)
    assert seg.shape == (L,)

    # ---- host-side preprocessing in float64 ----
    W = np.concatenate([np.asarray(Wd), np.asarray(Wa)], 0).astype(np.float64)
    b = np.concatenate([np.asarray(bd), np.asarray(ba)], 0).astype(np.float64)
    perm = np.argsort(seg, kind="stable")
    Wls = np.asarray(Wl).astype(np.float64)[perm]          # [1024, 2046]
    W1, W2 = Wls[:, :HID], Wls[:, HID:]
    V = 0.5 * (W1 - W2)
    W2s = 0.5 * (W1 + W2)
    G = (V @ W).T                                          # [128, 1024]
    c = V @ b                                              # [1024]

    counts = np.bincount(seg, minlength=NSEG)
    offs = np.concatenate([[0], np.cumsum(counts)])
    seg_bounds = tuple((int(offs[s]), int(offs[s + 1])) for s in range(NSEG))

    wt_np = np.zeros((P, KH), np.float16)
    wt_np[:, :HID] = W.T.astype(np.float16)
    bias_np = np.zeros(KH, np.float32)
    bias_np[:HID] = b
    bias_np[HID] = 1.0                                     # ones row -> pairs with c
    bias_np = np.ascontiguousarray(bias_np.reshape(KH // P, P).T)  # [128, 8]
    wg_np = np.zeros((KH // P + 1, P, L), np.float16)
    wg_np[0] = G.astype(np.float16)
    wl2sT = np.zeros((KH, L), np.float64)
    wl2sT[:HID] = W2s.T
    wl2sT[HID] = c
    wg_np[1:] = wl2sT.reshape(KH // P, P, L).astype(np.float16)

    nc = _get_program(seg.tobytes(), seg_bounds)

    in_maps = []
    ident_np = np.eye(P, dtype=np.float32)
    for i in range(N_CORES):
        in_maps.append({
            "x": np.ascontiguousarray(x[i * BC:(i + 1) * BC]),
            "wt": wt_np,
            "bias": bias_np,
            "wg": wg_np,
            "idin": ident_np,
        })

    global _last_in_maps
    _last_in_maps = in_maps

    res = run_bass_kernel_spmd(nc, in_maps, core_ids=list(range(N_CORES)))
    return np.concatenate([r["out"] for r in res.results], 0).astype(np.float32)


# revision 31
# speedup vs baseline: 1.0181x; 1.0181x over previous
"""Trainium2 Bass kernel for DTNetv1 (segment_reduce).

Reference computation:
    x1 = x @ Wd.T + bd ; x2 = x @ Wa.T + ba ; h = [x1, x2]            # [B, 1023]
    hh = [relu(h), relu(-h)]                                          # [B, 2046]
    leaf = hh @ Wl.T                                                  # [B, 1024]
    pooled = segment_max(leaf cols by seg_ids, 16) ; softmax(pooled)  # [B, 16]

Key algebraic rewrite (host-side, float64):
    relu(h) = (h + |h|)/2 ; relu(-h) = (|h| - h)/2
    leaf = h @ V.T + |h| @ W2s.T          V = (W1-W2)/2, W2s = (W1+W2)/2
         = x @ G + c + |h| @ W2s.T        G = W.T @ V.T (K=128!), c = V @ b
    This halves the dominant matmul contraction (2046 -> 1024+128).

Sharding: pure data-parallel over batch; 8 cores x 4096 rows, same NEFF.

Device pipeline per core (fp16 matmuls, fp32 PSUM accumulation):
    - PE-transpose x tiles -> xT [128(IN), B] fp16
    - stage A:  hT = W.T.T @ xT per 512-col group; ScalarE Abs(+bias) -> |h|T fp16
      (padded hidden row 1023 == 1.0 via bias, pairing with c packed in Wl2sT)
    - stage BC: leaf[128b, 1024] = xT_chunk.T @ G + sum_k |h|T_k.T @ Wl2sT_k
    - VectorE segment reduce_max over contiguous (pre-sorted) leaf columns
    - ScalarE Exp (+accumulated row sum), VectorE reciprocal + scale, DMA out

Measured (neuron-profile, core 0 of 8 running concurrently): ~166 us;
rel l2 error vs float64 oracle: 3.1e-4.
"""

import numpy as np

import concourse.bass as bass
import concourse.tile as tile
import concourse.mybir as mybir
from concourse import bacc
from concourse.bass_utils import run_bass_kernel_spmd

# Problem constants (hardcoded per the harness contract)
N_CORES = 8
B_FULL = 32768
IN = 128
D_DIM = 511
A_DIM = 512
HID = D_DIM + A_DIM          # 1023
KH = 1024                    # padded hidden (row 1023 is the "ones" bias row)
L = 1024                     # leaves
NSEG = 16
BC = B_FULL // N_CORES       # 4096 rows per core
GROUP = 512                  # batch rows per pipeline group
NSUB = 4                     # 128-row subchunks per group
NG = BC // GROUP             # 8 groups
P = 128

f32 = mybir.dt.float32
f16 = mybir.dt.float16


def _build(seg_bounds):
    """Build the Bass program. seg_bounds: list of (start, end) leaf-column
    ranges per segment in the sorted leaf order (end<=start for empty)."""
    nc = bacc.Bacc("TRN2", target_bir_lowering=False, debug=False)

    x = nc.dram_tensor("x", [BC, IN], f32, kind="ExternalInput").ap()
    wt = nc.dram_tensor("wt", [P, KH], f16, kind="ExternalInput").ap()
    bias = nc.dram_tensor("bias", [P, KH // P], f32, kind="ExternalInput").ap()
    wg = nc.dram_tensor("wg", [KH // P + 1, P, L], f16, kind="ExternalInput").ap()
    idin = nc.dram_tensor("idin", [P, P], f32, kind="ExternalInput").ap()
    out = nc.dram_tensor("out", [BC, NSEG], f32, kind="ExternalOutput").ap()

    x_v = x.rearrange("(g j p) i -> g p j i", g=NG, j=NSUB, p=P)
    out_v = out.rearrange("(g j p) s -> g j p s", g=NG, j=NSUB, p=P)
    wg_v = wg.rearrange("k p l -> p k l")

    NK = KH // P + 1  # 9 contraction chunks for stage BC (G + 8 |h| chunks)

    with tile.TileContext(nc) as tc:
        with (
            tc.tile_pool(name="consts", bufs=1) as consts,
            tc.tile_pool(name="xin", bufs=10) as xin_pool,
            tc.tile_pool(name="xts", bufs=3) as xt_pool,
            tc.tile_pool(name="hh", bufs=2) as hh_pool,
            tc.tile_pool(name="small", bufs=6) as small,
            tc.tile_pool(name="xtp", bufs=1, space="PSUM") as xt_ps,
            tc.tile_pool(name="aps", bufs=3, space="PSUM") as a_ps,
            tc.tile_pool(name="leafp", bufs=2, space="PSUM") as leaf_ps,
        ):
            xt_sb = [None] * NG   # [128, 512] fp16 xT per group
            hh_sb = [None] * NG   # [128, 8, 512] fp16 |h|T per group
            xin_t = [None] * NG

            def emit_load(g):
                # per-subchunk tiles/DMAs: first transpose only waits on 64KB
                xin_t[g] = []
                for c in range(NSUB):
                    t = xin_pool.tile([P, P], f32, tag="xin", name="xin")
                    nc.sync.dma_start(t, x_v[g][:, c, :])
                    xin_t[g].append(t)

            # group-0 x rows + identity land before the big weight streams
            # queue up — the first transposes are the startup critical path
            emit_load(0)
            ident = consts.tile([P, P], f32)
            nc.sync.dma_start(ident, idin)
            # PE warm-up: garbage matmuls during the startup DMA window keep
            # the HAM activity monitor busy so real matmuls start at 2.4 GHz.
            warm = consts.tile([P, GROUP], f16, tag="warm", name="warm")
            nc.gpsimd.memset(warm, 0.0)
            wps = a_ps.tile([P, GROUP], f32, tag="aps", name="wps")
            for _ in range(9):
                nc.tensor.matmul(wps, warm[:, 0:P], warm, start=True, stop=True)
            wt_t = consts.tile([P, KH], f16)
            nc.sync.dma_start(wt_t, wt)
            bias_t = consts.tile([P, KH // P], f32)
            nc.sync.dma_start(bias_t, bias)
            wg_t = []
            for k in range(NK):
                wg_t.append(consts.tile([P, L], f16, tag=f"wg{k}", name=f"wg{k}"))
                nc.sync.dma_start(wg_t[k], wg_v[:, k, :])

            def emit_transpose(g):
                # 4x PE transpose into one PSUM bank, one ACT evacuate+cast
                tps = xt_ps.tile([P, GROUP], f32, tag="xtp", name="xtp")
                for c in range(NSUB):
                    nc.tensor.transpose(
                        tps[:, c * P:(c + 1) * P], xin_t[g][c], ident
                    )
                xin_t[g] = None
                xt_sb[g] = xt_pool.tile([P, GROUP], f16, tag="xt", name="xt")
                nc.scalar.copy(xt_sb[g], tps)

            def emit_stage_a(g, c0, c1):
                # stage A matmuls for hidden chunks [c0, c1)
                for c in range(c0, c1):
                    aps = a_ps.tile([P, GROUP], f32, tag="aps", name="aps")
                    nc.tensor.matmul(
                        aps, wt_t[:, c * P:(c + 1) * P], xt_sb[g],
                        start=True, stop=True,
                    )
                    nc.scalar.activation(
                        hh_sb[g][:, c, :], aps,
                        mybir.ActivationFunctionType.Abs,
                        bias=bias_t[:, c:c + 1], scale=1.0,
                    )

            def lhsT_of(g, j, k):
                if k == 0:
                    return xt_sb[g][:, j * P:(j + 1) * P]
                return hh_sb[g][:, k - 1, j * P:(j + 1) * P]

            def emit_softmax_out(g, j, pooled):
                # softmax (pooled values are small; no max-subtraction needed)
                expt = small.tile([P, NSEG], f32, tag="expt", name="expt")
                sumt = small.tile([P, 1], f32, tag="sumt", name="sumt")
                nc.scalar.activation(
                    expt, pooled, mybir.ActivationFunctionType.Exp,
                    accum_out=sumt,
                )
                rect = small.tile([P, 1], f32, tag="rect", name="rect")
                nc.vector.reciprocal(rect, sumt)
                outt = small.tile([P, NSEG], f32, tag="outt", name="outt")
                nc.vector.tensor_scalar_mul(outt, expt, rect[:, 0:1])
                nc.sync.dma_start(out_v[g, j], outt)

            def emit_bc_sub(g, j):
                # leaf[128b, 1024] for subchunk j of group g
                leaf = leaf_ps.tile([P, L], f32, tag="leaf", name="leaf")
                pooled = small.tile([P, NSEG], f32, tag="pooled", name="pooled")
                if any(e <= s for s, e in seg_bounds):
                    nc.vector.memset(pooled, -1e30)
                for k in range(NK):
                    lhsT = lhsT_of(g, j, k)
                    nc.tensor.matmul(
                        leaf[:, 0:512], lhsT, wg_t[k][:, 0:512],
                        start=(k == 0), stop=(k == NK - 1),
                    )
                    nc.tensor.matmul(
                        leaf[:, 512:L], lhsT, wg_t[k][:, 512:L],
                        start=(k == 0), stop=(k == NK - 1),
                    )
                # evacuate PSUM via ScalarE (frees the banks fast), then
                # ragged per-segment max over contiguous column ranges
                leaf_sb = small.tile([P, L], f32, tag="leafsb", name="leaf_sb")
                nc.scalar.copy(leaf_sb, leaf)
                for s, (lo, hi) in enumerate(seg_bounds):
                    if hi > lo:
                        nc.vector.reduce_max(
                            out=pooled[:, s:s + 1], in_=leaf_sb[:, lo:hi],
                            axis=mybir.AxisListType.X,
                        )
                emit_softmax_out(g, j, pooled)

            def emit_bc_sub_last(g, j):
                # final subchunk: two independent PSUM tiles so bank-0
                # segment reduces overlap bank-1 matmuls, shrinking the
                # serial reduce chain on the kernel tail
                poolA = small.tile([P, NSEG], f32, tag="pooled", name="poolA")
                poolB = small.tile([P, NSEG], f32, tag="poolB", name="poolB")
                nc.vector.memset(poolA, -1e30)
                nc.vector.memset(poolB, -1e30)
                leafA = leaf_ps.tile([P, L], f32, tag="leaf", name="leafA")
                for k in range(NK):
                    nc.tensor.matmul(
                        leafA[:, 0:512], lhsT_of(g, j, k), wg_t[k][:, 0:512],
                        start=(k == 0), stop=(k == NK - 1),
                    )
                leafB = leaf_ps.tile([P, L], f32, tag="leaf", name="leafB")
                for s, (lo, hi) in enumerate(seg_bounds):
                    if min(hi, 512) > lo:
                        nc.vector.reduce_max(
                            out=poolA[:, s:s + 1], in_=leafA[:, lo:min(hi, 512)],
                            axis=mybir.AxisListType.X,
                        )
                for k in range(NK):
                    nc.tensor.matmul(
                        leafB[:, 0:512], lhsT_of(g, j, k), wg_t[k][:, 512:L],
                        start=(k == 0), stop=(k == NK - 1),
                    )
                for s, (lo, hi) in enumerate(seg_bounds):
                    if hi - 512 > max(lo - 512, 0):
                        nc.vector.reduce_max(
                            out=poolB[:, s:s + 1],
                            in_=leafB[:, max(lo - 512, 0):hi - 512],
                            axis=mybir.AxisListType.X,
                        )
                nc.vector.tensor_tensor(
                    poolA, poolA, poolB, mybir.AluOpType.max
                )
                emit_softmax_out(g, j, poolA)

            # --- pipelined emission ---
            for gi in range(NG + 1):
                if gi + 1 < NG:
                    emit_load(gi + 1)
                if gi < NG:
                    emit_transpose(gi)
                    if gi == 0:
                        # bridge the HAM activity gap: transposes don't count
                        # as PE-busy, so without these the clock re-throttles
                        # right as stage A starts (trace: K=4/8 dip at ~16us)
                        wps2 = a_ps.tile([P, GROUP], f32, tag="aps", name="wps2")
                        for _ in range(5):
                            nc.tensor.matmul(
                                wps2, warm[:, 0:P], warm, start=True, stop=True
                            )
                    hh_sb[gi] = hh_pool.tile([P, KH // P, GROUP], f16, tag="hh", name="hh")
                # interleave stage A of group gi with stage BC of group gi-1
                for j in range(NSUB):
                    if gi >= 1:
                        if gi == NG and j == NSUB - 1:
                            emit_bc_sub_last(gi - 1, j)
                        else:
                            emit_bc_sub(gi - 1, j)
                    if gi < NG:
                        emit_stage_a(gi, 2 * j, 2 * j + 2)
                if gi >= 1:
                    xt_sb[gi - 1] = None
                    hh_sb[gi - 1] = None

    nc.compile()
    return nc


_cache = {}
_last_in_maps = None


def _get_program(seg_key, seg_bounds):
    if seg_key not in _cache:
        _cache[seg_key] = _build(seg_bounds)
    return _cache[seg_key]


def kernel(x, Wd, bd, Wa, ba, Wl, seg_ids):
    x = np.ascontiguousarray(np.asarray(x, dtype=np.float32))
    seg = np.asarray(seg_ids).astype(np.int64)
    assert x.shape == (B_FULL, IN), x.shape
    assert np.asarray(Wl).shape == (L, 2 * HID)
    assert seg.shape == (L,)

    # ---- host-side preprocessing in float64 ----
    W = np.concatenate([np.asarray(Wd), np.asarray(Wa)], 0).astype(np.float64)
    b = np.concatenate([np.asarray(bd), np.asarray(ba)], 0).astype(np.float64)
    perm = np.argsort(seg, kind="stable")
    Wls = np.asarray(Wl).astype(np.float64)[perm]          # [1024, 2046]
    W1, W2 = Wls[:, :HID], Wls[:, HID:]
    V = 0.5 * (W1 - W2)
    W2s = 0.5 * (W1 + W2)
    G = (V @ W).T                                          # [128, 1024]
    c = V @ b                                              # [1024]

    counts = np.bincount(seg, minlength=NSEG)
    offs = np.concatenate([[0], np.cumsum(counts)])
    seg_bounds = tuple((int(offs[s]), int(offs[s + 1])) for s in range(NSEG))

    wt_np = np.zeros((P, KH), np.float16)
    wt_np[:, :HID] = W.T.astype(np.float16)
    bias_np = np.zeros(KH, np.float32)
    bias_np[:HID] = b
    bias_np[HID] = 1.0                                     # ones row -> pairs with c
    bias_np = np.ascontiguousarray(bias_np.reshape(KH // P, P).T)  # [128, 8]
    wg_np = np.zeros((KH // P + 1, P, L), np.float16)
    wg_np[0] = G.astype(np.float16)
    wl2sT = np.zeros((KH, L), np.float64)
    wl2sT[:HID] = W2s.T
    wl2sT[HID] = c
    wg_np[1:] = wl2sT.reshape(KH // P, P, L).astype(np.float16)

    nc = _get_program(seg.tobytes(), seg_bounds)

    in_maps = []
    ident_np = np.eye(P, dtype=np.float32)
    for i in range(N_CORES):
        in_maps.append({
            "x": np.ascontiguousarray(x[i * BC:(i + 1) * BC]),
            "wt": wt_np,
            "bias": bias_np,
            "wg": wg_np,
            "idin": ident_np,
        })

    global _last_in_maps
    _last_in_maps = in_maps

    res = run_bass_kernel_spmd(nc, in_maps, core_ids=list(range(N_CORES)))
    return np.concatenate([r["out"] for r in res.results], 0).astype(np.float32)


# revision 37
# speedup vs baseline: 1.0453x; 1.0268x over previous
"""Trainium2 Bass kernel for DTNetv1 (segment_reduce).

Reference computation:
    x1 = x @ Wd.T + bd ; x2 = x @ Wa.T + ba ; h = [x1, x2]            # [B, 1023]
    hh = [relu(h), relu(-h)]                                          # [B, 2046]
    leaf = hh @ Wl.T                                                  # [B, 1024]
    pooled = segment_max(leaf cols by seg_ids, 16) ; softmax(pooled)  # [B, 16]

Key algebraic rewrite (host-side, float64):
    relu(h) = (h + |h|)/2 ; relu(-h) = (|h| - h)/2
    leaf = h @ V.T + |h| @ W2s.T          V = (W1-W2)/2, W2s = (W1+W2)/2
         = x @ G + c + |h| @ W2s.T        G = W.T @ V.T (K=128!), c = V @ b
    This halves the dominant matmul contraction (2046 -> 1024+128).

Sharding: pure data-parallel over batch; 8 cores x 4096 rows, same NEFF.

Device pipeline per core (fp16 matmuls, fp32 PSUM accumulation):
    - x.T arrives pre-transposed/pre-cast fp16 from the host
    - stage A:  hT = W.T.T @ xT per 512-col group; ScalarE Abs(+bias) -> |h|T fp16
      (padded hidden row 1023 == 1.0 via bias, pairing with c packed in Wl2sT)
    - stage BC: leaf[128b, 1024] = xT_chunk.T @ G + sum_k |h|T_k.T @ Wl2sT_k
    - VectorE segment reduce_max over contiguous (pre-sorted) leaf columns
    - ScalarE Exp (+accumulated row sum), VectorE reciprocal + scale, DMA out

Measured (neuron-profile, core 0 of 8 running concurrently): ~166 us;
rel l2 error vs float64 oracle: 3.1e-4.
"""

import numpy as np

import concourse.bass as bass
import concourse.tile as tile
import concourse.mybir as mybir
from concourse import bacc
from concourse.bass_utils import run_bass_kernel_spmd

# Problem constants (hardcoded per the harness contract)
N_CORES = 8
B_FULL = 32768
IN = 128
D_DIM = 511
A_DIM = 512
HID = D_DIM + A_DIM          # 1023
KH = 1024                    # padded hidden (row 1023 is the "ones" bias row)
L = 1024                     # leaves
NSEG = 16
BC = B_FULL // N_CORES       # 4096 rows per core
GROUP = 512                  # batch rows per pipeline group
NSUB = 4                     # 128-row subchunks per group
NG = BC // GROUP             # 8 groups
P = 128

f32 = mybir.dt.float32
f16 = mybir.dt.float16


def _build(seg_bounds):
    """Build the Bass program. seg_bounds: list of (start, end) leaf-column
    ranges per segment in the sorted leaf order (end<=start for empty)."""
    nc = bacc.Bacc("TRN2", target_bir_lowering=False, debug=False)

    xt = nc.dram_tensor("xt", [P, BC], f16, kind="ExternalInput").ap()
    wt = nc.dram_tensor("wt", [P, KH], f16, kind="ExternalInput").ap()
    bias = nc.dram_tensor("bias", [P, KH // P], f32, kind="ExternalInput").ap()
    wg = nc.dram_tensor("wg", [KH // P + 1, P, L], f16, kind="ExternalInput").ap()
    out = nc.dram_tensor("out", [BC, NSEG], f32, kind="ExternalOutput").ap()

    out_v = out.rearrange("(g j p) s -> g j p s", g=NG, j=NSUB, p=P)
    wg_v = wg.rearrange("k p l -> p k l")

    NK = KH // P + 1  # 9 contraction chunks for stage BC (G + 8 |h| chunks)

    with tile.TileContext(nc) as tc:
        with (
            tc.tile_pool(name="consts", bufs=1) as consts,
            tc.tile_pool(name="xts", bufs=3) as xt_pool,
            tc.tile_pool(name="hh", bufs=2) as hh_pool,
            tc.tile_pool(name="small", bufs=6) as small,
            tc.tile_pool(name="aps", bufs=4, space="PSUM") as a_ps,
            tc.tile_pool(name="leafp", bufs=2, space="PSUM") as leaf_ps,
        ):
            xt_sb = [None] * NG   # [128, 512] fp16 xT per group
            hh_sb = [None] * NG   # [128, 8, 512] fp16 |h|T per group

            def emit_load(g):
                # x.T arrives pre-transposed/pre-cast from the host
                xt_sb[g] = xt_pool.tile([P, GROUP], f16, tag="xt", name="xt")
                nc.sync.dma_start(xt_sb[g], xt[:, g * GROUP:(g + 1) * GROUP])

            # group-0 x.T lands before the big weight streams queue up —
            # the first stage-A matmuls are the startup critical path
            emit_load(0)
            # PE warm-up: garbage matmuls during the startup DMA window keep
            # the HAM activity monitor busy so real matmuls start at 2.4 GHz.
            warm = consts.tile([P, GROUP], f16, tag="warm", name="warm")
            nc.gpsimd.memset(warm, 0.0)
            wps = a_ps.tile([P, GROUP], f32, tag="aps", name="wps")
            for _ in range(9):
                nc.tensor.matmul(wps, warm[:, 0:P], warm, start=True, stop=True)
            wt_t = consts.tile([P, KH], f16)
            nc.sync.dma_start(wt_t, wt)
            bias_t = consts.tile([P, KH // P], f32)
            nc.sync.dma_start(bias_t, bias)
            wg_t = []
            for k in range(NK):
                wg_t.append(consts.tile([P, L], f16, tag=f"wg{k}", name=f"wg{k}"))
                nc.sync.dma_start(wg_t[k], wg_v[:, k, :])

            def emit_stage_a(g, c0, c1):
                # stage A matmuls for hidden chunks [c0, c1)
                for c in range(c0, c1):
                    aps = a_ps.tile([P, GROUP], f32, tag="aps", name="aps")
                    nc.tensor.matmul(
                        aps, wt_t[:, c * P:(c + 1) * P], xt_sb[g],
                        start=True, stop=True,
                    )
                    nc.scalar.activation(
                        hh_sb[g][:, c, :], aps,
                        mybir.ActivationFunctionType.Abs,
                        bias=bias_t[:, c:c + 1], scale=1.0,
                    )

            def lhsT_of(g, j, k):
                if k == 0:
                    return xt_sb[g][:, j * P:(j + 1) * P]
                return hh_sb[g][:, k - 1, j * P:(j + 1) * P]

            def emit_softmax_out(g, j, pooled):
                # softmax (pooled values are small; no max-subtraction needed)
                expt = small.tile([P, NSEG], f32, tag="expt", name="expt")
                sumt = small.tile([P, 1], f32, tag="sumt", name="sumt")
                nc.scalar.activation(
                    expt, pooled, mybir.ActivationFunctionType.Exp,
                    accum_out=sumt,
                )
                rect = small.tile([P, 1], f32, tag="rect", name="rect")
                nc.vector.reciprocal(rect, sumt)
                outt = small.tile([P, NSEG], f32, tag="outt", name="outt")
                nc.vector.tensor_scalar_mul(outt, expt, rect[:, 0:1])
                nc.sync.dma_start(out_v[g, j], outt)

            def emit_bc_sub(g, j):
                # leaf[128b, 1024] for subchunk j of group g
                leaf = leaf_ps.tile([P, L], f32, tag="leaf", name="leaf")
                pooled = small.tile([P, NSEG], f32, tag="pooled", name="pooled")
                if any(e <= s for s, e in seg_bounds):
                    nc.vector.memset(pooled, -1e30)
                for k in range(NK):
                    lhsT = lhsT_of(g, j, k)
                    nc.tensor.matmul(
                        leaf[:, 0:512], lhsT, wg_t[k][:, 0:512],
                        start=(k == 0), stop=(k == NK - 1),
                    )
                    nc.tensor.matmul(
                        leaf[:, 512:L], lhsT, wg_t[k][:, 512:L],
                        start=(k == 0), stop=(k == NK - 1),
                    )
                # evacuate PSUM via ScalarE (frees the banks fast), then
                # ragged per-segment max over contiguous column ranges
                leaf_sb = small.tile([P, L], f32, tag="leafsb", name="leaf_sb")
                nc.scalar.copy(leaf_sb, leaf)
                for s, (lo, hi) in enumerate(seg_bounds):
                    if hi > lo:
                        nc.vector.reduce_max(
                            out=pooled[:, s:s + 1], in_=leaf_sb[:, lo:hi],
                            axis=mybir.AxisListType.X,
                        )
                emit_softmax_out(g, j, pooled)

            def emit_bc_sub_last(g, j):
                # final subchunk: two independent PSUM tiles so bank-0
                # segment reduces overlap bank-1 matmuls, shrinking the
                # serial reduce chain on the kernel tail
                poolA = small.tile([P, NSEG], f32, tag="pooled", name="poolA")
                poolB = small.tile([P, NSEG], f32, tag="poolB", name="poolB")
                nc.vector.memset(poolA, -1e30)
                nc.vector.memset(poolB, -1e30)
                leafA = leaf_ps.tile([P, L], f32, tag="leaf", name="leafA")
                for k in range(NK):
                    nc.tensor.matmul(
                        leafA[:, 0:512], lhsT_of(g, j, k), wg_t[k][:, 0:512],
                        start=(k == 0), stop=(k == NK - 1),
                    )
                leafB = leaf_ps.tile([P, L], f32, tag="leaf", name="leafB")
                for s, (lo, hi) in enumerate(seg_bounds):
                    if min(hi, 512) > lo:
                        nc.vector.reduce_max(
                            out=poolA[:, s:s + 1], in_=leafA[:, lo:min(hi, 512)],
                            axis=mybir.AxisListType.X,
                        )
                for k in range(NK):
                    nc.tensor.matmul(
                        leafB[:, 0:512], lhsT_of(g, j, k), wg_t[k][:, 512:L],
                        start=(k == 0), stop=(k == NK - 1),
                    )
                for s, (lo, hi) in enumerate(seg_bounds):
                    if hi - 512 > max(lo - 512, 0):
                        nc.vector.reduce_max(
                            out=poolB[:, s:s + 1],
                            in_=leafB[:, max(lo - 512, 0):hi - 512],
                            axis=mybir.AxisListType.X,
                        )
                nc.vector.tensor_tensor(
                    poolA, poolA, poolB, mybir.AluOpType.max
                )
                emit_softmax_out(g, j, poolA)

            # --- pipelined emission ---
            for gi in range(NG + 1):
                if gi + 1 < NG:
                    emit_load(gi + 1)
                if gi < NG:
                    hh_sb[gi] = hh_pool.tile([P, KH // P, GROUP], f16, tag="hh", name="hh")
                # interleave stage A of group gi with stage BC of group gi-1
                for j in range(NSUB):
                    if gi >= 1:
                        if gi == NG and j == NSUB - 1:
                            emit_bc_sub_last(gi - 1, j)
                        else:
                            emit_bc_sub(gi - 1, j)
                    if gi < NG:
                        emit_stage_a(gi, 2 * j, 2 * j + 2)
                if gi >= 1:
                    xt_sb[gi - 1] = None
                    hh_sb[gi - 1] = None

    nc.compile()
    return nc


_cache = {}
_last_in_maps = None


def _get_program(seg_key, seg_bounds):
    if seg_key not in _cache:
        _cache[seg_key] = _build(seg_bounds)
    return _cache[seg_key]


def kernel(x, Wd, bd, Wa, ba, Wl, seg_ids):
    x = np.ascontiguousarray(np.asarray(x, dtype=np.float32))
    seg = np.asarray(seg_ids).astype(np.int64)
    assert x.shape == (B_FULL, IN), x.shape
    assert np.asarray(Wl).shape == (L, 2 * HID)
    assert seg.shape == (L,)

    # ---- host-side preprocessing in float64 ----
    W = np.concatenate([np.asarray(Wd), np.asarray(Wa)], 0).astype(np.float64)
    b = np.concatenate([np.asarray(bd), np.asarray(ba)], 0).astype(np.float64)
    perm = np.argsort(seg, kind="stable")
    Wls = np.asarray(Wl).astype(np.float64)[perm]          # [1024, 2046]
    W1, W2 = Wls[:, :HID], Wls[:, HID:]
    V = 0.5 * (W1 - W2)
    W2s = 0.5 * (W1 + W2)
    G = (V @ W).T                                          # [128, 1024]
    c = V @ b                                              # [1024]

    counts = np.bincount(seg, minlength=NSEG)
    offs = np.concatenate([[0], np.cumsum(counts)])
    seg_bounds = tuple((int(offs[s]), int(offs[s + 1])) for s in range(NSEG))

    wt_np = np.zeros((P, KH), np.float16)
    wt_np[:, :HID] = W.T.astype(np.float16)
    bias_np = np.zeros(KH, np.float32)
    bias_np[:HID] = b
    bias_np[HID] = 1.0                                     # ones row -> pairs with c
    bias_np = np.ascontiguousarray(bias_np.reshape(KH // P, P).T)  # [128, 8]
    wg_np = np.zeros((KH // P + 1, P, L), np.float16)
    wg_np[0] = G.astype(np.float16)
    wl2sT = np.zeros((KH, L), np.float64)
    wl2sT[:HID] = W2s.T
    wl2sT[HID] = c
    wg_np[1:] = wl2sT.reshape(KH // P, P, L).astype(np.float16)

    nc = _get_program(seg.tobytes(), seg_bounds)

    in_maps = []
    xt_np = x.T.astype(np.float16)          # [128, 32768], pre-transposed
    for i in range(N_CORES):
        in_maps.append({
            "xt": np.ascontiguousarray(xt_np[:, i * BC:(i + 1) * BC]),
            "wt": wt_np,
            "bias": bias_np,
            "wg": wg_np,
        })

    global _last_in_maps
    _last_in_maps = in_maps

    res = run_bass_kernel_spmd(nc, in_maps, core_ids=list(range(N_CORES)))
    return np.concatenate([r["out"] for r in res.results], 0).astype(np.float32)


# revision 39
# speedup vs baseline: 1.0552x; 1.0094x over previous
"""Trainium2 Bass kernel for DTNetv1 (segment_reduce).

Reference computation:
    x1 = x @ Wd.T + bd ; x2 = x @ Wa.T + ba ; h = [x1, x2]            # [B, 1023]
    hh = [relu(h), relu(-h)]                                          # [B, 2046]
    leaf = hh @ Wl.T                                                  # [B, 1024]
    pooled = segment_max(leaf cols by seg_ids, 16) ; softmax(pooled)  # [B, 16]

Key algebraic rewrite (host-side, float64):
    relu(h) = (h + |h|)/2 ; relu(-h) = (|h| - h)/2
    leaf = h @ V.T + |h| @ W2s.T          V = (W1-W2)/2, W2s = (W1+W2)/2
         = x @ G + c + |h| @ W2s.T        G = W.T @ V.T (K=128!), c = V @ b
    This halves the dominant matmul contraction (2046 -> 1024+128).

Sharding: pure data-parallel over batch; 8 cores x 4096 rows, same NEFF.

Device pipeline per core (fp16 matmuls, fp32 PSUM accumulation):
    - x.T arrives pre-transposed/pre-cast fp16 from the host
    - stage A:  hT = W.T.T @ xT per 512-col group; ScalarE Abs(+bias) -> |h|T fp16
      (padded hidden row 1023 == 1.0 via bias, pairing with c packed in Wl2sT)
    - stage BC: leaf[128b, 1024] = xT_chunk.T @ G + sum_k |h|T_k.T @ Wl2sT_k
    - VectorE segment reduce_max over contiguous (pre-sorted) leaf columns
    - ScalarE Exp (+accumulated row sum), VectorE reciprocal + scale, DMA out

Measured (neuron-profile, core 0 of 8 running concurrently): ~166 us;
rel l2 error vs float64 oracle: 3.1e-4.
"""

import numpy as np

import concourse.bass as bass
import concourse.tile as tile
import concourse.mybir as mybir
from concourse import bacc
from concourse.bass_utils import run_bass_kernel_spmd

# Problem constants (hardcoded per the harness contract)
N_CORES = 8
B_FULL = 32768
IN = 128
D_DIM = 511
A_DIM = 512
HID = D_DIM + A_DIM          # 1023
KH = 1024                    # padded hidden (row 1023 is the "ones" bias row)
L = 1024                     # leaves
NSEG = 16
BC = B_FULL // N_CORES       # 4096 rows per core
GROUP = 512                  # batch rows per pipeline group
NSUB = 4                     # 128-row subchunks per group
NG = BC // GROUP             # 8 groups
P = 128

f32 = mybir.dt.float32
f16 = mybir.dt.float16


def _build(seg_bounds):
    """Build the Bass program. seg_bounds: list of (start, end) leaf-column
    ranges per segment in the sorted leaf order (end<=start for empty)."""
    nc = bacc.Bacc("TRN2", target_bir_lowering=False, debug=False)

    xt = nc.dram_tensor("xt", [P, BC], f16, kind="ExternalInput").ap()
    wt = nc.dram_tensor("wt", [P, KH], f16, kind="ExternalInput").ap()
    bias = nc.dram_tensor("bias", [P, KH // P], f32, kind="ExternalInput").ap()
    wg = nc.dram_tensor("wg", [KH // P + 1, P, L], f16, kind="ExternalInput").ap()
    out = nc.dram_tensor("out", [BC, NSEG], f32, kind="ExternalOutput").ap()

    out_v = out.rearrange("(g j p) s -> g j p s", g=NG, j=NSUB, p=P)
    wg_v = wg.rearrange("k p l -> p k l")

    NK = KH // P + 1  # 9 contraction chunks for stage BC (G + 8 |h| chunks)

    with tile.TileContext(nc) as tc:
        with (
            tc.tile_pool(name="consts", bufs=1) as consts,
            tc.tile_pool(name="xts", bufs=5) as xt_pool,
            tc.tile_pool(name="hh", bufs=3) as hh_pool,
            tc.tile_pool(name="small", bufs=6) as small,
            tc.tile_pool(name="aps", bufs=4, space="PSUM") as a_ps,
            tc.tile_pool(name="leafp", bufs=2, space="PSUM") as leaf_ps,
        ):
            xt_sb = [None] * NG   # [128, 512] fp16 xT per group
            hh_sb = [None] * NG   # [128, 8, 512] fp16 |h|T per group

            def emit_load(g):
                # x.T arrives pre-transposed/pre-cast from the host
                xt_sb[g] = xt_pool.tile([P, GROUP], f16, tag="xt", name="xt")
                nc.sync.dma_start(xt_sb[g], xt[:, g * GROUP:(g + 1) * GROUP])

            # group-0 x.T lands before the big weight streams queue up —
            # the first stage-A matmuls are the startup critical path
            emit_load(0)
            # PE warm-up: garbage matmuls during the startup DMA window keep
            # the HAM activity monitor busy so real matmuls start at 2.4 GHz.
            warm = consts.tile([P, GROUP], f16, tag="warm", name="warm")
            nc.gpsimd.memset(warm, 0.0)
            wps = a_ps.tile([P, GROUP], f32, tag="aps", name="wps")
            for _ in range(9):
                nc.tensor.matmul(wps, warm[:, 0:P], warm, start=True, stop=True)
            wt_t = consts.tile([P, KH], f16)
            nc.sync.dma_start(wt_t, wt)
            bias_t = consts.tile([P, KH // P], f32)
            nc.sync.dma_start(bias_t, bias)
            wg_t = []
            for k in range(NK):
                wg_t.append(consts.tile([P, L], f16, tag=f"wg{k}", name=f"wg{k}"))
                nc.sync.dma_start(wg_t[k], wg_v[:, k, :])

            def emit_stage_a(g, c0, c1):
                # stage A matmuls for hidden chunks [c0, c1)
                for c in range(c0, c1):
                    aps = a_ps.tile([P, GROUP], f32, tag="aps", name="aps")
                    nc.tensor.matmul(
                        aps, wt_t[:, c * P:(c + 1) * P], xt_sb[g],
                        start=True, stop=True,
                    )
                    nc.scalar.activation(
                        hh_sb[g][:, c, :], aps,
                        mybir.ActivationFunctionType.Abs,
                        bias=bias_t[:, c:c + 1], scale=1.0,
                    )

            def lhsT_of(g, j, k):
                if k == 0:
                    return xt_sb[g][:, j * P:(j + 1) * P]
                return hh_sb[g][:, k - 1, j * P:(j + 1) * P]

            def emit_softmax_out(g, j, pooled):
                # softmax (pooled values are small; no max-subtraction needed)
                expt = small.tile([P, NSEG], f32, tag="expt", name="expt")
                sumt = small.tile([P, 1], f32, tag="sumt", name="sumt")
                nc.scalar.activation(
                    expt, pooled, mybir.ActivationFunctionType.Exp,
                    accum_out=sumt,
                )
                rect = small.tile([P, 1], f32, tag="rect", name="rect")
                nc.vector.reciprocal(rect, sumt)
                outt = small.tile([P, NSEG], f32, tag="outt", name="outt")
                nc.vector.tensor_scalar_mul(outt, expt, rect[:, 0:1])
                nc.sync.dma_start(out_v[g, j], outt)

            def emit_bc_sub(g, j):
                # leaf[128b, 1024] for subchunk j of group g
                leaf = leaf_ps.tile([P, L], f32, tag="leaf", name="leaf")
                pooled = small.tile([P, NSEG], f32, tag="pooled", name="pooled")
                if any(e <= s for s, e in seg_bounds):
                    nc.vector.memset(pooled, -1e30)
                for k in range(NK):
                    lhsT = lhsT_of(g, j, k)
                    nc.tensor.matmul(
                        leaf[:, 0:512], lhsT, wg_t[k][:, 0:512],
                        start=(k == 0), stop=(k == NK - 1),
                    )
                    nc.tensor.matmul(
                        leaf[:, 512:L], lhsT, wg_t[k][:, 512:L],
                        start=(k == 0), stop=(k == NK - 1),
                    )
                # evacuate PSUM via ScalarE (frees the banks fast), then
                # ragged per-segment max over contiguous column ranges
                leaf_sb = small.tile([P, L], f32, tag="leafsb", name="leaf_sb")
                nc.scalar.copy(leaf_sb, leaf)
                for s, (lo, hi) in enumerate(seg_bounds):
                    if hi > lo:
                        nc.vector.reduce_max(
                            out=pooled[:, s:s + 1], in_=leaf_sb[:, lo:hi],
                            axis=mybir.AxisListType.X,
                        )
                emit_softmax_out(g, j, pooled)

            def emit_bc_sub_last(g, j):
                # final subchunk: two independent PSUM tiles so bank-0
                # segment reduces overlap bank-1 matmuls, shrinking the
                # serial reduce chain on the kernel tail
                poolA = small.tile([P, NSEG], f32, tag="pooled", name="poolA")
                poolB = small.tile([P, NSEG], f32, tag="poolB", name="poolB")
                nc.vector.memset(poolA, -1e30)
                nc.vector.memset(poolB, -1e30)
                leafA = leaf_ps.tile([P, L], f32, tag="leaf", name="leafA")
                for k in range(NK):
                    nc.tensor.matmul(
                        leafA[:, 0:512], lhsT_of(g, j, k), wg_t[k][:, 0:512],
                        start=(k == 0), stop=(k == NK - 1),
                    )
                leafB = leaf_ps.tile([P, L], f32, tag="leaf", name="leafB")
                for s, (lo, hi) in enumerate(seg_bounds):
                    if min(hi, 512) > lo:
                        nc.vector.reduce_max(
                            out=poolA[:, s:s + 1], in_=leafA[:, lo:min(hi, 512)],
                            axis=mybir.AxisListType.X,
                        )
                for k in range(NK):
                    nc.tensor.matmul(
                        leafB[:, 0:512], lhsT_of(g, j, k), wg_t[k][:, 512:L],
                        start=(k == 0), stop=(k == NK - 1),
                    )
                for s, (lo, hi) in enumerate(seg_bounds):
                    if hi - 512 > max(lo - 512, 0):
                        nc.vector.reduce_max(
                            out=poolB[:, s:s + 1],
                            in_=leafB[:, max(lo - 512, 0):hi - 512],
                            axis=mybir.AxisListType.X,
                        )
                nc.vector.tensor_tensor(
                    poolA, poolA, poolB, mybir.AluOpType.max
                )
                emit_softmax_out(g, j, poolA)

            # --- pipelined emission ---
            # stage BC lags stage A by TWO groups: A needs only wt+xt (land
            # early), BC needs the big wg stream — the lag turns early BC
            # DMA-arrival stalls into useful A work.
            LAG = 2
            for gi in range(NG + LAG):
                if gi + 1 < NG:
                    emit_load(gi + 1)
                if gi < NG:
                    hh_sb[gi] = hh_pool.tile([P, KH // P, GROUP], f16, tag="hh", name="hh")
                for j in range(NSUB):
                    if gi >= LAG:
                        if gi == NG + LAG - 1 and j == NSUB - 1:
                            emit_bc_sub_last(gi - LAG, j)
                        else:
                            emit_bc_sub(gi - LAG, j)
                    if gi < NG:
                        emit_stage_a(gi, 2 * j, 2 * j + 2)
                if gi >= LAG:
                    xt_sb[gi - LAG] = None
                    hh_sb[gi - LAG] = None

    nc.compile()
    return nc


_cache = {}
_last_in_maps = None


def _get_program(seg_key, seg_bounds):
    if seg_key not in _cache:
        _cache[seg_key] = _build(seg_bounds)
    return _cache[seg_key]


def kernel(x, Wd, bd, Wa, ba, Wl, seg_ids):
    x = np.ascontiguousarray(np.asarray(x, dtype=np.float32))
    seg = np.asarray(seg_ids).astype(np.int64)
    assert x.shape == (B_FULL, IN), x.shape
    assert np.asarray(Wl).shape == (L, 2 * HID)
    assert seg.shape == (L,)

    # ---- host-side preprocessing in float64 ----
    W = np.concatenate([np.asarray(Wd), np.asarray(Wa)], 0).astype(np.float64)
    b = np.concatenate([np.asarray(bd), np.asarray(ba)], 0).astype(np.float64)
    perm = np.argsort(seg, kind="stable")
    Wls = np.asarray(Wl).astype(np.float64)[perm]          # [1024, 2046]
    W1, W2 = Wls[:, :HID], Wls[:, HID:]
    V = 0.5 * (W1 - W2)
    W2s = 0.5 * (W1 + W2)
    G = (V @ W).T                                          # [128, 1024]
    c = V @ b                                              # [1024]

    counts = np.bincount(seg, minlength=NSEG)
    offs = np.concatenate([[0], np.cumsum(counts)])
    seg_bounds = tuple((int(offs[s]), int(offs[s + 1])) for s in range(NSEG))

    wt_np = np.zeros((P, KH), np.float16)
    wt_np[:, :HID] = W.T.astype(np.float16)
    bias_np = np.zeros(KH, np.float32)
    bias_np[:HID] = b
    bias_np[HID] = 1.0                                     # ones row -> pairs with c
    bias_np = np.ascontiguousarray(bias_np.reshape(KH // P, P).T)  # [128, 8]
    wg_np = np.zeros((KH // P + 1, P, L), np.float16)
    wg_np[0] = G.astype(np.float16)
    wl2sT = np.zeros((KH, L), np.float64)
    wl2sT[:HID] = W2s.T
    wl2sT[HID] = c
    wg_np[1:] = wl2sT.reshape(KH // P, P, L).astype(np.float16)

    nc = _get_program(seg.tobytes(), seg_bounds)

    in_maps = []
    xt_np = x.T.astype(np.float16)          # [128, 32768], pre-transposed
    for i in range(N_CORES):
        in_maps.append({
            "xt": np.ascontiguousarray(xt_np[:, i * BC:(i + 1) * BC]),
            "wt": wt_np,
            "bias": bias_np,
            "wg": wg_np,
        })

    global _last_in_maps
    _last_in_maps = in_maps

    res = run_bass_kernel_spmd(nc, in_maps, core_ids=list(range(N_CORES)))
    return np.concatenate([r["out"] for r in res.results], 0).astype(np.float32)
